# revision 1
# baseline (speedup 1.0000x reference)
"""Trainium2 8-core SPMD kernel for nn_AdaptiveReluMPNN (entry: kernel()).

Strategy: dst-range node sharding across 8 cores; per layer each core
computes its message-table rows (M = h @ Wlin.T + blin), AllGather ->
full table [R=102400, 64] f32.  The table is split in 4 windows of 25600
rows (dma_gather int16 reach); each core's in-edges are grouped by
(dst-node, window) into K-padded groups (duplicate-first padding),
bucketed by K.  dma_gather pulls 256B rows chunk-wise (edge-major),
TensorE transposes into feature-major PSUM chunks [128, WSL] holding two
64-feature column sets; DVE 3D-AP tensor_reduce yields per-group
min/max/sum; group stats go to per-window DRAM stat tables; a second
dma_gather aligns them per node and TT trees combine the <=4 window
partials.  bias = t*mx + (1-t)*mn.  Sweep 2 re-reads spilled bf16
feature-major chunks, computes relu(m - bias) (bias delivered
feature-major via transpose-mode dma_gather of a node bias table) and
reduces to relu_sum.  agg combines the 5 stats with baked scalar weights;
h_next = Wc @ h + bc + agg on TensorE/ACT/DVE.  The global pooling phase
reuses the same machinery over batch-groups split by window, and a final
small matmul emits out[16, 32] per core.

All indices/layout are compile-time constants (NEFF specialized per
input graph); float tensors are runtime inputs.
"""
import sys

sys.path.insert(0, "/opt/trn_rl_repo")

import numpy as np

N, E, F, G, OUT, L = 100000, 1600000, 64, 128, 32, 2
M = 8
NWIN = 4
KSET = (2, 4, 6, 8, 12, 16, 24, 32, 48, 64, 96, 128)
WSL = 1536                 # slots per chunk half (all K in KSET divide it)
NC = 12800                 # padded nodes per core
R = M * NC
WROWS = R // NWIN          # 25600 < 32767
NT = NC // 128             # 100


def _lcm(a, b):
    return int(np.lcm(a, b))




def _fm_slots(s0, half, fmc):
    """slot positions for fm columns `fmc` on partition-half `half` of a
    chunk starting at slot s0 (adjacent-piece-pair interleaved layout)."""
    fmc = np.asarray(fmc)
    return s0 + (fmc // 128) * 256 + half * 128 + (fmc % 128)

class Plan:
    def __init__(self, edge_index, batch):
        src = np.asarray(edge_index[0], np.int64)
        dst = np.asarray(edge_index[1], np.int64)
        batch = np.asarray(batch, np.int64)
        deg = np.bincount(dst, minlength=N)
        assert deg.min() >= 1 and deg.max() <= max(KSET)
        cum = np.cumsum(deg)
        gsz = np.bincount(batch, minlength=G)
        gptr = np.concatenate([[0], np.cumsum(gsz)])   # group node boundaries
        bounds = [0]
        for i in range(1, M):
            tgt = int(np.searchsorted(cum, E * i / M))
            gi = int(np.argmin(np.abs(gptr - tgt)))
            bounds.append(int(gptr[gi]))
        bounds.append(N)
        self.bounds = np.array(bounds)
        self.gbounds = np.searchsorted(gptr, bounds)   # group index per bound
        assert (gptr[self.gbounds] == np.array(bounds)).all()
        self.n_real = self.bounds[1:] - self.bounds[:-1]
        assert self.n_real.max() <= NC

        pos = np.full(N, -1, np.int64)
        for c in range(M):
            lo, hi = bounds[c], bounds[c + 1]
            pos[lo:hi] = c * NC + np.arange(hi - lo)
        self.pos = pos

        win_of = pos[src] // WROWS
        order = np.lexsort((win_of, dst))
        src_s, win_s = src[order], win_of[order]
        rowptr = np.concatenate([[0], cum])
        rows_s = pos[src_s]                          # table row of each edge

        dw = np.zeros((N, NWIN), np.int64)
        np.add.at(dw, (dst, win_of), 1)
        dwcum = np.concatenate([np.zeros((N, 1), np.int64),
                                np.cumsum(dw, axis=1)], 1)
        Ks = np.array(KSET)
        kidx = np.searchsorted(Ks, dw)

        glists = {}
        for c in range(M):
            lo, hi = bounds[c], bounds[c + 1]
            nodes = np.arange(lo, hi)
            for w in range(NWIN):
                d = dw[lo:hi, w]
                for b in range(len(KSET)):
                    glists[(c, w, b)] = nodes[(d > 0) & (kidx[lo:hi, w] == b)]

        self.gcount = {}
        for w in range(NWIN):
            for b, K in enumerate(KSET):
                g = max(len(glists[(c, w, b)]) for c in range(M))
                q = 2 * _lcm(128, K) // K
                self.gcount[(w, b)] = ((g + q - 1) // q) * q if g else 0

        # chunks: (w, b, K, slot0, paircol0, ncols); bucket chunk bases
        self.chunks = []
        self.bucket_paircol0 = {}
        self.bucket_chunk_slot0 = {}
        self.win_paircol0 = np.zeros(NWIN + 1, np.int64)
        slot, pc = 0, 0
        for w in range(NWIN):
            self.win_paircol0[w] = pc
            for b, K in enumerate(KSET):
                gc = self.gcount[(w, b)]
                if gc == 0:
                    continue
                self.bucket_paircol0[(w, b)] = pc
                cols_b = gc // 2
                step = (WSL // _lcm(128, K)) * (_lcm(128, K) // K)
                c0, bases = 0, []
                while c0 < cols_b:
                    nc_ = min(step, cols_b - c0)
                    self.chunks.append((w, b, K, slot, pc + c0, nc_))
                    bases.append((c0, nc_, slot))
                    slot += 2 * nc_ * K
                    c0 += nc_
                self.bucket_chunk_slot0[(w, b)] = (step, bases)
                pc += cols_b
        self.win_paircol0[NWIN] = pc
        self.S, self.PC = slot, pc
        self.PCpad = -(-pc // 128) * 128
        self.win_rows = [int(2 * (self.win_paircol0[w + 1] - self.win_paircol0[w]))
                         for w in range(NWIN)]

        self.slot_idx = np.zeros((M, self.S), np.int16)   # window-relative row
        self.slot_win = np.zeros(self.S, np.int8)         # window per slot (layout)
        self.pad_arr = np.zeros((M, 2, self.PC), np.float32)
        self.cnt = np.zeros((M, NC), np.float32)
        self.comb_idx = {}                                # (c,w) -> [NC] rows
        for c in range(M):
            for w in range(NWIN):
                self.comb_idx[(c, w)] = np.full(NC, self.win_rows[w], np.int64)
            self.cnt[c, :self.n_real[c]] = deg[bounds[c]:bounds[c + 1]]
        self.biasA_idx = np.zeros((M, self.PCpad), np.int64)
        self.biasB_idx = np.zeros((M, self.PCpad), np.int64)

        for (w, b), gc in self.gcount.items():
            if gc == 0:
                continue
            K = KSET[b]
            cols_b = gc // 2
            pc0 = self.bucket_paircol0[(w, b)]
            step, bases = self.bucket_chunk_slot0[(w, b)]
            # group slot0 for every col j in bucket (vectorized)
            gj = np.arange(gc)
            half = (gj >= cols_b).astype(np.int64)
            col = gj - half * cols_b
            ch = col // step
            c_in = col - ch * step
            ncs = np.array([nc_ for (_, nc_, _) in bases])
            s0s = np.array([s for (_, _, s) in bases])
            # fm-column start of each group within its chunk
            fmc0 = c_in * K
            # mark layout window for all slots of this bucket
            for (_, nc_, s) in bases:
                self.slot_win[s:s + 2 * nc_ * K] = w
            for c in range(M):
                nodes = glists[(c, w, b)]
                n_g = len(nodes)
                if n_g == 0:
                    continue
                d = dw[nodes, w]
                st = rowptr[nodes] + dwcum[nodes, w]
                # fill slots (vectorized ragged)
                k_ = np.tile(np.arange(K), n_g)
                d_ = np.repeat(d, K)
                st_ = np.repeat(st, K)
                eidx = st_ + k_ * (k_ < d_)
                fmc_ = np.repeat(fmc0[:n_g], K) + k_
                sl = _fm_slots(np.repeat(s0s[ch[:n_g]], K),
                               np.repeat(half[:n_g], K), fmc_)
                self.slot_idx[c, sl] = (rows_s[eidx] - w * WROWS).astype(np.int16)
                hj, cj = half[:n_g], col[:n_g]
                paircol = pc0 + cj
                self.pad_arr[c, hj, paircol] = K - d
                npos = pos[nodes] - c * NC
                wl = paircol - self.win_paircol0[w]
                self.comb_idx[(c, w)][npos] = 2 * wl + hj
                a = hj == 0
                self.biasA_idx[c, paircol[a]] = npos[a]
                self.biasB_idx[c, paircol[~a]] = npos[~a]

        n = np.arange(NC)
        self.nrow = (n % 128) * NT + n // 128             # node -> table row

        # ---------------- pooling (local-table, group-per-core) ----------
        self.KG = 896                        # max group size 880 <= 896 = 7*128
        assert gsz.max() <= self.KG
        self.GPMAX = int(np.max(np.diff(self.gbounds)))
        if self.GPMAX % 2:
            self.GPMAX += 1
        self.GPCOL = self.GPMAX // 2
        KG = self.KG
        self.SG = 2 * self.GPCOL * KG
        self.gslot_idx = np.zeros((M, self.SG), np.int16)
        self.gpad = np.zeros((M, 2, self.GPCOL), np.float32)
        self.gcnt = np.zeros((M, 2, self.GPCOL), np.float32)
        self.gnum = np.diff(self.gbounds)    # real groups per core
        for c in range(M):
            g0, g1 = self.gbounds[c], self.gbounds[c + 1]
            for j, gid in enumerate(range(g0, g1)):
                npos = np.arange(gptr[gid], gptr[gid + 1]) - bounds[c]
                d = len(npos)
                assert 0 < d <= KG
                half, col = (0, j) if j < self.GPCOL else (1, j - self.GPCOL)
                sl = _fm_slots(col * 2 * KG, half, np.arange(KG))
                self.gslot_idx[c, sl[:d]] = npos.astype(np.int16)
                self.gslot_idx[c, sl[d:]] = npos[0]
                self.gpad[c, half, col] = KG - d
                self.gcnt[c, half, col] = d

def wrap_idx16(raw):
    n = len(raw)
    assert n % 16 == 0
    w = np.zeros((16, n // 16), np.int16)
    w[np.arange(n) % 16, np.arange(n) // 16] = raw.astype(np.int16)
    return np.tile(w, (8, 1))


# ---------------------------------------------------------------- mirror
def mirror(inputs, plan):
    """Numpy mirror of the device computation (per-core, canonical layout)."""
    x = np.asarray(inputs["x"], np.float32)
    Wlin = np.asarray(inputs["Wlin"], np.float32)
    blin = np.asarray(inputs["blin"], np.float32)
    t = np.clip(np.asarray(inputs["t"], np.float32), 0, 1)
    Wproj = np.asarray(inputs["Wproj"], np.float32)
    bproj = np.asarray(inputs["bproj"], np.float32)
    Wc = np.asarray(inputs["Wc"], np.float32)
    bc = np.asarray(inputs["bc"], np.float32)
    gW = np.asarray(inputs["gWlin"], np.float32)
    gb = np.asarray(inputs["gblin"], np.float32)
    gt = np.clip(np.asarray(inputs["gt"], np.float32), 0, 1)
    gWp = np.asarray(inputs["gWproj"], np.float32)
    gbp = np.asarray(inputs["gbproj"], np.float32)
    Wout = np.asarray(inputs["Wout"], np.float32)
    bout = np.asarray(inputs["bout"], np.float32)

    h = []
    for c in range(M):
        hc = np.zeros((NC, 64), np.float32)
        nr = plan.n_real[c]
        hc[:nr] = x[plan.bounds[c]:plan.bounds[c + 1]]
        h.append(hc)

    def edge_phase(table, slot_idx, slot_win, chunks, pad_arr, comb, nwinrows,
                   tt, c):
        """Returns node-aligned (mn, mx, sm, rs) [NC?, 64] given table and
        per-slot layout; comb(w) -> [n] stat rows; generic over edge/pooling
        (for pooling comb is trivial)."""
        absrow = slot_idx.astype(np.int64) + slot_win.astype(np.int64) * WROWS
        vals = table[absrow]                           # [S, 64]
        stats = {}                                     # (w): [rows, 64] per st
        for w in range(NWIN):
            stats[w] = {s: np.zeros((nwinrows[w] + 1, 64), np.float32)
                        for s in ("mn", "mx", "sm", "fr")}
            stats[w]["mn"][nwinrows[w]] = 1e30
            stats[w]["mx"][nwinrows[w]] = -1e30
        for (w, b, K, s0, pc0, ncols) in chunks:
            wl0 = pc0 - plan.win_paircol0[w]
            for half in range(2):
                sl = _fm_slots(s0, half, np.arange(ncols * K))
                v = vals[sl].reshape(ncols, K, 64)
                mn, mx = v.min(1), v.max(1)
                sm, first = v.sum(1), v[:, 0]
                pd = pad_arr[half, pc0:pc0 + ncols][:, None]
                sm = sm - pd * first
                rr = 2 * (wl0 + np.arange(ncols)) + half
                stats[w]["mn"][rr] = mn
                stats[w]["mx"][rr] = mx
                stats[w]["sm"][rr] = sm
                stats[w]["fr"][rr] = first
        return vals, stats

    def combine(stats, comb, nwinrows):
        mn = np.full((len(comb[(0)]), 64), 1e30, np.float32) if False else None
        # node-aligned combine
        n = comb[0].shape[0]
        mn = np.full((n, 64), 1e30, np.float32)
        mx = np.full((n, 64), -1e30, np.float32)
        sm = np.zeros((n, 64), np.float32)
        for w in range(NWIN):
            rows = comb[w]
            mn = np.minimum(mn, stats[w]["mn"][rows])
            mx = np.maximum(mx, stats[w]["mx"][rows])
            smw = stats[w]["sm"].copy()
            smw[nwinrows[w]] = 0
            sm = sm + smw[rows]
        return mn, mx, sm

    def relu_phase(vals, bias_of_slotgroup, chunks, pad_arr, nwinrows):
        stats = {w: np.zeros((nwinrows[w] + 1, 64), np.float32)
                 for w in range(NWIN)}
        for (w, b, K, s0, pc0, ncols) in chunks:
            wl0 = pc0 - plan.win_paircol0[w]
            for half in range(2):
                sl = _fm_slots(s0, half, np.arange(ncols * K))
                v = vals[sl].reshape(ncols, K, 64)
                bia = bias_of_slotgroup(w, half, pc0, ncols)   # [ncols, 64]
                r = np.maximum(v - bia[:, None, :], 0)
                rs = r.sum(1)
                first = r[:, 0]
                pd = pad_arr[half, pc0:pc0 + ncols][:, None]
                rs = rs - pd * first
                rr = 2 * (wl0 + np.arange(ncols)) + half
                stats[w][rr] = rs
        return stats

    g_nwinrows = None

    for l in range(L):
        tabs = [h[c] @ Wlin[l].T + blin[l] for c in range(M)]
        table = np.concatenate(tabs, 0)
        newh = []
        for c in range(M):
            comb = {w: plan.comb_idx[(c, w)] for w in range(NWIN)}
            vals, stats = edge_phase(table, plan.slot_idx[c], plan.slot_win,
                                     plan.chunks, plan.pad_arr[c], comb,
                                     plan.win_rows, t[l], c)
            mn, mx, sm = combine(stats, comb, plan.win_rows)
            bias = t[l][None, :] * mx + (1 - t[l][None, :]) * mn   # [NC, 64]

            def bias_of(w, half, pc0, ncols, c=c, bias=bias):
                idx = (plan.biasA_idx if half == 0 else plan.biasB_idx)[
                    c, pc0:pc0 + ncols]
                return bias[idx].astype(np.float32)

            rstats = relu_phase(vals, bias_of, plan.chunks, plan.pad_arr[c],
                                plan.win_rows)
            rs = np.zeros((NC, 64), np.float32)
            for w in range(NWIN):
                rw = rstats[w].copy()
                rw[plan.win_rows[w]] = 0
                rs = rs + rw[comb[w]]
            wp = Wproj[l]
            agg = (wp[0] * plan.cnt[c][:, None] + wp[1] * mn + wp[2] * mx
                   + wp[3] * rs + wp[4] * sm + bproj[l])
            newh.append(h[c] @ Wc[l].T + bc[l] + agg)
        h = newh

    # pooling: per-core local table (each core owns whole groups)
    outs = []
    KG, GPCOL = plan.KG, plan.GPCOL
    for c in range(M):
        loc = np.zeros((NC, 64), np.float32)
        loc[:, :] = h[c] @ gW.T + gb
        vals = loc[plan.gslot_idx[c].astype(np.int64)]     # [SG, 64]
        v = np.zeros((2, GPCOL, KG, 64), np.float32)
        for half in range(2):
            for col in range(GPCOL):
                sl = _fm_slots(col * 2 * KG, half, np.arange(KG))
                v[half, col] = vals[sl]
        mn, mx = v.min(2), v.max(2)
        first = v[:, :, 0]
        pd = plan.gpad[c][:, :, None]
        sm = v.sum(2) - pd * first
        bias = gt[None, None] * mx + (1 - gt[None, None]) * mn  # [2, GPCOL, 64]
        r = np.maximum(v - bias[:, :, None, :], 0)
        rs = r.sum(2) - pd * r[:, :, 0]
        emb = (gWp[0] * plan.gcnt[c][:, :, None] + gWp[1] * mn + gWp[2] * mx
               + gWp[3] * rs + gWp[4] * sm + gbp)       # [2, GPCOL, 64]
        emb = emb.reshape(2 * GPCOL, 64)
        o = emb @ Wout.T + bout                          # [GPMAX, OUT]
        outs.append(o[:plan.gnum[c]])
    return np.concatenate(outs, 0)





def _patch_tile_swdge_lanes():
    """Make Tile's SWDGE completion-sem lane == the instruction's SWDGE
    queue_num (required for multi-queue dma_gather: each DMASW sem lane must
    be fed by exactly one queue)."""
    from concourse import tile_sem_assignment as tsa
    if getattr(tsa, "_lane_eq_queue", False):
        return
    tsa._lane_eq_queue = True
    import concourse.mybir as mybir

    orig = tsa.TileClockTick._assign_tick

    def _assign_tick(self, inst):
        if (isinstance(inst, tsa.DMAInst)
                and inst.engine == mybir.EngineType.Pool):
            q = getattr(inst, "queue_num", None)
            self.next_sw_dma_idx = int(q) if q is not None else 0
        return orig(self, inst)

    tsa.TileClockTick._assign_tick = _assign_tick

# ================================================================= builder
def build_nc(plan, dbg=False):
    _patch_tile_swdge_lanes()
    from concourse import bass, mybir, bacc, tile
    from concourse.masks import make_identity

    f32 = mybir.dt.float32
    bf16 = mybir.dt.bfloat16
    i16 = mybir.dt.int16
    ALU = mybir.AluOpType
    AFT = mybir.ActivationFunctionType
    AX = mybir.AxisListType

    S, PC, KG, GPCOL = plan.S, plan.PC, plan.KG, plan.GPCOL
    SG = plan.SG
    wr = plan.win_rows
    wincols = [int(plan.win_paircol0[w + 1] - plan.win_paircol0[w])
               for w in range(NWIN)]
    PCW = [-(-wc // 128) * 128 for wc in wincols]
    maxpcw = max(PCW)
    consts = plan.consts

    nc = bacc.Bacc("TRN2", target_bir_lowering=False, debug=dbg, num_devices=M,
                   num_swdge_queues=4,
                   dynamic_dma_scratch_size=28672)
    _q = [0]

    def qn():
        v = _q[0] % 4
        _q[0] += 1
        return v

    def din(name, shape, dt=f32):
        return nc.dram_tensor(name, shape, dt, kind="ExternalInput")

    x_in = din("x", [64, NC])
    sidx = din("sidx", [128, S // 16], i16)
    cidx = [din(f"cidx{w}", [128, NC // 16], i16) for w in range(NWIN)]
    baidx = din("baidx", [128, sum(PCW) // 16], i16)
    bbidx = din("bbidx", [128, sum(PCW) // 16], i16)
    gsidx = din("gsidx", [128, SG // 16], i16)
    padf = din("padf", [128, PC])
    cntb = din("cntb", [128, NT])
    tb = [din(f"tb{l}", [128, 64]) for l in range(L)]
    gtb = din("gtb", [128, 1])
    wlin = [din(f"wlin{l}", [65, 64]) for l in range(L)]
    wcm = [din(f"wc{l}", [64, 64]) for l in range(L)]
    bcv = [din(f"bc{l}", [64, 1]) for l in range(L)]
    gwa = din("gwa", [65, 64])
    wouta = din("wouta", [65, 32])
    gpadt = din("gpadt", [128, GPCOL])
    c0g = din("c0g", [128, GPCOL])
    out_ext = nc.dram_tensor("out", [plan.GPMAX, OUT], f32,
                             kind="ExternalOutput")

    with tile.TileContext(nc) as tc:
        with (
            tc.tile_pool(name="persist", bufs=1) as pp,
            tc.tile_pool(name="sweep", bufs=2) as sw,
            tc.tile_pool(name="cg", bufs=2) as cgp,
            tc.tile_pool(name="stats", bufs=1) as stp,
            tc.tile_pool(name="small", bufs=2) as smp,
            tc.tile_pool(name="psum", bufs=2, space="PSUM") as psp,
            tc.tile_pool(name="psum1", bufs=2, space="PSUM") as ps1,
            tc.tile_pool(name="dram", bufs=1, space="DRAM") as dp,
        ):
            # ---------------- persistent SBUF ----------------
            ident = pp.tile([128, 128], f32, tag="ident")
            make_identity(nc, ident[:, :])
            cnt_t = pp.tile([128, NT], f32, tag="cnt")
            nc.sync.dma_start(out=cnt_t[:, :], in_=cntb[:, :])
            t_t = [pp.tile([128, 64], f32, tag=f"t{l}", name=f"t_t{l}") for l in range(L)]
            wlin_t = [pp.tile([65, 64], f32, tag=f"wl{l}", name=f"wlin_t{l}") for l in range(L)]
            wc_t = [pp.tile([64, 64], f32, tag=f"wct{l}", name=f"wc_t{l}") for l in range(L)]
            bc_t = [pp.tile([64, 1], f32, tag=f"bct{l}", name=f"bc_t{l}") for l in range(L)]
            for l in range(L):
                nc.sync.dma_start(out=t_t[l][:, :], in_=tb[l][:, :])
                nc.sync.dma_start(out=wlin_t[l][:, :], in_=wlin[l][:, :])
                nc.sync.dma_start(out=wc_t[l][:, :], in_=wcm[l][:, :])
                nc.sync.dma_start(out=bc_t[l][:, :], in_=bcv[l][:, :])
            gwa_t = pp.tile([65, 64], f32, tag="gwa")
            wout_t = pp.tile([65, 32], f32, tag="wout")
            gtb_t = pp.tile([128, 1], f32, tag="gtb")
            gpad_t = pp.tile([128, GPCOL], f32, tag="gpadT")
            c0g_t = pp.tile([128, GPCOL], f32, tag="c0gT")
            nc.sync.dma_start(out=gwa_t[:, :], in_=gwa[:, :])
            nc.sync.dma_start(out=wout_t[:, :], in_=wouta[:, :])
            nc.sync.dma_start(out=gtb_t[:, :], in_=gtb[:, :])
            nc.sync.dma_start(out=gpad_t[:, :], in_=gpadt[:, :])
            nc.sync.dma_start(out=c0g_t[:, :], in_=c0g[:, :])

            # ---------------- DRAM internals ----------------
            h_d = dp.tile([64, NC], f32)
            nc.sync.dma_start(out=h_d[:, :], in_=x_in[:, :])
            m_my = dp.tile([NC, 64], f32)
            table = dp.tile([R, 64], f32)
            spill = dp.tile([128, S // 2], bf16)
            bias_nm = dp.tile([128, NT * 128], bf16)
            stat_nm = {}
            for st in ("mn", "mx", "sm", "rs"):
                for w in range(NWIN):
                    stat_nm[(st, w)] = dp.tile([wr[w] + 2, 64], f32, name=f"stat_{st}{w}")
            sentp = smp.tile([1, 64], f32, tag="sentp")
            nc.vector.memset(sentp[:, :], 1e30)
            sentn = smp.tile([1, 64], f32, tag="sentn")
            nc.vector.memset(sentn[:, :], -1e30)
            zs = smp.tile([1, 64], f32, tag="zsent")
            nc.vector.memset(zs[:, :], 0.0)
            for w in range(NWIN):
                nc.sync.dma_start(out=stat_nm[("mn", w)][wr[w]:wr[w] + 1, :],
                                  in_=sentp[:, :])
                nc.sync.dma_start(out=stat_nm[("mx", w)][wr[w]:wr[w] + 1, :],
                                  in_=sentn[:, :])
                nc.sync.dma_start(out=stat_nm[("sm", w)][wr[w]:wr[w] + 1, :],
                                  in_=zs[0:1, :])
                nc.sync.dma_start(out=stat_nm[("rs", w)][wr[w]:wr[w] + 1, :],
                                  in_=zs[0:1, :])

            def mtable_phase(wt, dst_dram):
                bs = min(8, NT)
                for j0 in range(0, NT, bs):
                    je = min(j0 + bs, NT)
                    mst = smp.tile([128, bs * 64], f32, tag="mst")
                    for j in range(j0, je):
                        hst = sw.tile([65, 128], f32, tag="hst")
                        nc.vector.memset(hst[64:65, :], 1.0)
                        nc.sync.dma_start(out=hst[0:64, :],
                                          in_=h_d[:, 128 * j:128 * (j + 1)])
                        ps = ps1.tile([128, 64], f32, tag="ps_small")
                        nc.tensor.matmul(out=ps[:, :], lhsT=hst[:, :],
                                         rhs=wt[:, :], start=True, stop=True)
                        nc.scalar.copy(out=mst[:, 64 * (j - j0):64 * (j - j0 + 1)],
                                       in_=ps[:, :])
                    nc.sync.dma_start(
                        out=dst_dram[:, :]
                        .rearrange("(j p) f -> p j f", p=128)[:, j0:je, :],
                        in_=mst[:, :(je - j0) * 64])

            def stat_flush(st, w, arr, pcl, ncols):
                """arr [128, ncols] fm chunk-stats -> stat table rows."""
                for p0 in range(0, ncols, 128):
                    n_ = min(128, ncols - p0)
                    ps = ps1.tile([128, 128], f32, tag="ps_small")
                    nc.tensor.transpose(out=ps[:n_, :], in_=arr[:, p0:p0 + n_],
                                        identity=ident[:, :])
                    stg = smp.tile([128, 128], f32, tag="ststage")
                    nc.scalar.copy(out=stg[:n_, :], in_=ps[:n_, :])
                    r0 = 2 * (pcl + p0)
                    nc.sync.dma_start(
                        out=stat_nm[(st, w)][r0:r0 + 2 * n_, :]
                        .rearrange("(a b) f -> a (b f)", b=2),
                        in_=stg[:n_, :])

            def comb_gather(st, accum_fn, only_w=None):
                for w in ((only_w,) if only_w is not None else range(NWIN)):
                    for hh in range(2):
                        n0 = hh * (NC // 2)
                        ct = sw.tile([128, NC // 32], i16, tag="cidx")
                        nc.sync.dma_start(
                            out=ct[:, :],
                            in_=cidx[w][:, n0 // 16:(n0 + NC // 2) // 16])
                        gt_ = cgp.tile([128, NT // 2, 64], f32, tag="cgat",
                                       name="cgat")
                        nc.gpsimd.dma_gather(
                            out_ap=gt_[:, :, :], in_ap=stat_nm[(st, w)][:, :],
                            idxs_ap=ct[:, :], num_idxs=NC // 2,
                            num_idxs_reg=NC // 2, elem_size=64,
                            single_packet=False, queue_num=qn())
                        accum_fn(w, hh,
                                 gt_[:, :, :].rearrange("p a b -> p (a b)"))

            def bfm_build(bfm, w):
                off = sum(PCW[:w]) // 16
                nn = PCW[w]
                for half, bix in ((0, baidx), (1, bbidx)):
                    ix = sw.tile([128, maxpcw // 16], i16, tag="bidx")
                    nc.sync.dma_start(out=ix[:, :nn // 16],
                                      in_=bix[:, off:off + nn // 16])
                    bt = (bfm if half == 0 else
                          stp.tile([128, maxpcw], bf16, tag="stC", name="btB"))
                    nc.gpsimd.dma_gather(
                        out_ap=bt[:, None, :nn],
                        in_ap=bias_nm[:, :].rearrange("p (a b) -> (p a) b",
                                                      b=128),
                        idxs_ap=ix[:, :nn // 16], num_idxs=nn, num_idxs_reg=nn,
                        elem_size=128, transpose=True, single_packet=False, queue_num=qn())
                    if half == 1:
                        nc.sync.dma_start(out=bfm[64:128, :nn],
                                          in_=bt[0:64, :nn])

            def sweep(second, bfm=None, win_done=None):
                cur_w = -1
                for (w, b, K, s0, pc0, ncols) in plan.chunks:
                    if w != cur_w:
                        if second:
                            if cur_w >= 0 and win_done is not None:
                                win_done(cur_w)
                            bfm_build(bfm, w)
                        cur_w = w
                    nsl = 2 * ncols * K
                    npc = nsl // 128
                    pcl = int(pc0 - plan.win_paircol0[w])
                    cst = {}
                    if not second:
                        it = sw.tile([128, nsl // 16], i16, tag="idx")
                        nc.sync.dma_start(
                            out=it[:, :],
                            in_=sidx[:, s0 // 16:(s0 + nsl) // 16])
                        g = sw.tile([128, npc, 64], f32, tag="gat", bufs=3)
                        nc.gpsimd.dma_gather(
                            out_ap=g[:, :, :],
                            in_ap=table[w * WROWS:(w + 1) * WROWS, :],
                            idxs_ap=it[:, :], num_idxs=nsl, num_idxs_reg=nsl,
                            elem_size=64, single_packet=False, queue_num=qn())
                        fm = psp.tile([128, nsl // 2], f32, tag="fm")
                        gp = g[:, :, :].rearrange("p a b -> p (a b)")
                        for i in range(npc // 2):
                            nc.tensor.transpose(
                                out=fm[:, 128 * i:128 * i + 128],
                                in_=gp[:, 128 * i:128 * i + 128],
                                identity=ident[:, :])
                        v3 = fm[:, :].rearrange("p (n k) -> p n k", k=K)
                        for st, op in (("mn", ALU.min), ("mx", ALU.max),
                                       ("sm", ALU.add)):
                            cst[st] = sw.tile([128, ncols], f32, tag=f"c_{st}", name=f"cst_{st}")
                            nc.vector.tensor_reduce(out=cst[st][:, :], in_=v3,
                                                    axis=AX.X, op=op)
                        src_first = fm[:, ::K]
                        corr = cst["sm"]
                        sp = sw.tile([128, nsl // 2], bf16, tag="spl")
                        nc.scalar.copy(out=sp[:, :], in_=fm[:, :])
                        nc.sync.dma_start(
                            out=spill[:, s0 // 2:s0 // 2 + nsl // 2],
                            in_=sp[:, :])
                    else:
                        rt = sw.tile([128, nsl // 2], bf16, tag="rld")
                        nc.sync.dma_start(
                            out=rt[:, :],
                            in_=spill[:, s0 // 2:s0 // 2 + nsl // 2])
                        rl = sw.tile([128, nsl // 2], f32, tag="relu")
                        bsl = bfm[:, pcl:pcl + ncols, None] \
                            .to_broadcast([128, ncols, K])
                        nc.vector.tensor_tensor(
                            out=rl[:, :].rearrange("p (n k) -> p n k", k=K),
                            in0=rt[:, :].rearrange("p (n k) -> p n k", k=K),
                            in1=bsl, op=ALU.subtract)
                        nc.vector.tensor_scalar(
                            out=rl[:, :], in0=rl[:, :], scalar1=0.0,
                            scalar2=None, op0=ALU.max)
                        cst["rs"] = sw.tile([128, ncols], f32, tag="c_rs", name="cst_rs")
                        nc.vector.tensor_reduce(
                            out=cst["rs"][:, :],
                            in_=rl[:, :].rearrange("p (n k) -> p n k", k=K),
                            axis=AX.X, op=ALU.add)
                        src_first = rl[:, ::K]
                        corr = cst["rs"]
                    pdt = sw.tile([128, ncols], f32, tag="pad")
                    nc.sync.dma_start(out=pdt[:, :],
                                      in_=padf[:, pc0:pc0 + ncols])
                    fw = sw.tile([128, ncols], f32, tag="fw")
                    nc.vector.tensor_tensor(out=fw[:, :], in0=src_first,
                                            in1=pdt[:, :], op=ALU.mult)
                    nc.vector.tensor_tensor(out=corr[:, :], in0=corr[:, :],
                                            in1=fw[:, :], op=ALU.subtract)
                    for st in cst:
                        stat_flush(st, w, cst[st], pcl, ncols)

            # ================= layers =================
            for l in range(L):
                wp = consts["Wproj"][l]
                mtable_phase(wlin_t[l], m_my)
                nc.gpsimd.collective_compute(
                    "AllGather", ALU.bypass,
                    ins=[m_my[:, :].opt()], outs=[table[:, :].opt()],
                    replica_groups=[list(range(M))])
                sweep(False)

                A = stp.tile([128, NT * 64], f32, tag="stA")   # mn -> agg
                B = stp.tile([128, NT * 64], f32, tag="stB")   # mx
                C = stp.tile([128, NT * 64], f32, tag="stC")   # bias

                def fold(dst, w, hh, ap, op):
                    d = dst[:, hh * (NT // 2) * 64:
                            (hh * (NT // 2) + NT // 2) * 64]
                    if w == 0:
                        nc.vector.tensor_copy(d, ap)
                    else:
                        nc.vector.tensor_tensor(out=d, in0=d, in1=ap, op=op)

                comb_gather("mn", lambda w, hh, ap: fold(A, w, hh, ap, ALU.min))
                comb_gather("mx", lambda w, hh, ap: fold(B, w, hh, ap, ALU.max))
                # bias = t*(mx-mn) + mn  -> C
                nc.vector.tensor_tensor(out=C[:, :], in0=B[:, :], in1=A[:, :],
                                        op=ALU.subtract)
                tbc = t_t[l][:, None, :].to_broadcast([128, NT, 64])
                C3 = C[:, :].rearrange("p (a b) -> p a b", b=64)
                nc.vector.tensor_tensor(out=C3, in0=C3, in1=tbc,
                                        op=ALU.mult)
                nc.vector.tensor_tensor(out=C[:, :], in0=C[:, :], in1=A[:, :],
                                        op=ALU.add)
                # agg (into A): A = w1*mn; += w2*mx; += w0*cnt + bproj
                nc.vector.tensor_scalar(out=A[:, :], in0=A[:, :],
                                        scalar1=float(wp[1]), scalar2=None,
                                        op0=ALU.mult)
                nc.vector.scalar_tensor_tensor(
                    out=A[:, :], in0=B[:, :], scalar=float(wp[2]), in1=A[:, :],
                    op0=ALU.mult, op1=ALU.add)
                cbc = cnt_t[:, :, None].to_broadcast([128, NT, 64])
                A3 = A[:, :].rearrange("p (a b) -> p a b", b=64)
                nc.vector.scalar_tensor_tensor(
                    out=A3, in0=cbc, scalar=float(wp[0]), in1=A3,
                    op0=ALU.mult, op1=ALU.add)
                nc.vector.tensor_scalar(out=A[:, :], in0=A[:, :],
                                        scalar1=float(consts["bproj"][l]),
                                        scalar2=None, op0=ALU.add)
                # bias -> bf16 (DVE cast) -> node table (HWDGE DMA)
                bbf = stp.tile([128, NT * 64], bf16, tag="stB")
                nc.vector.tensor_copy(bbf[:, :], C[:, :])
                nc.sync.dma_start(
                    out=bias_nm[:, :]
                    .rearrange("p (a b) -> p a b", b=128)[:, :, 0:64],
                    in_=bbf[:, :].rearrange("p (a b) -> p a b", b=64))

                def foldw(w, hh, ap, wgt):
                    d = A[:, hh * (NT // 2) * 64:
                          (hh * (NT // 2) + NT // 2) * 64]
                    nc.vector.scalar_tensor_tensor(
                        out=d, in0=ap, scalar=float(wgt), in1=d,
                        op0=ALU.mult, op1=ALU.add)

                comb_gather("sm", lambda w, hh, ap: foldw(w, hh, ap, wp[4]))

                bfm = stp.tile([128, maxpcw], bf16, tag="stB")

                def rs_done(wd):
                    comb_gather("rs",
                                lambda w, hh, ap: foldw(w, hh, ap, wp[3]),
                                only_w=wd)

                sweep(True, bfm=bfm, win_done=rs_done)
                rs_done(NWIN - 1)

                # h = Wc @ h + bc + agg(A)
                aggv = A[:, :].rearrange("p (a b) -> p a b", b=64)
                for j in range(NT):
                    hst = sw.tile([65, 128], f32, tag="hst")
                    nc.sync.dma_start(out=hst[0:64, :],
                                      in_=h_d[:, 128 * j:128 * (j + 1)])
                    psA = ps1.tile([64, 128], f32, tag="ps_small")
                    nc.tensor.matmul(out=psA[:, :], lhsT=wc_t[l][:, :],
                                     rhs=hst[0:64, :],
                                     start=True, stop=True)
                    psB = ps1.tile([64, 128], f32, tag="ps_small")
                    nc.tensor.transpose(out=psB[:, :], in_=aggv[:, j, :],
                                        identity=ident[:, :])
                    ht = smp.tile([64, 128], f32, tag="ht")
                    nc.scalar.activation(out=ht[:, :], in_=psA[:, :],
                                         func=AFT.Identity, bias=bc_t[l][:, :])
                    ht2 = smp.tile([64, 128], f32, tag="ht2")
                    nc.vector.tensor_tensor(out=ht2[:, :], in0=ht[:, :],
                                            in1=psB[:, :], op=ALU.add)
                    nc.sync.dma_start(out=h_d[:, 128 * j:128 * (j + 1)],
                                      in_=ht2[:, :])

            # ================= pooling =================
            mtable_phase(gwa_t, m_my)
            gwp = consts["gWp"]
            gstat = {st: smp.tile([128, GPCOL], f32, tag=f"g_{st}", name=f"gstat_{st}")
                     for st in ("mn", "mx", "sm", "rs")}
            gfirst = smp.tile([128, GPCOL], f32, tag="g_fr")
            gbias = smp.tile([128, GPCOL], f32, tag="g_bias")
            for phase in range(2):
                for pcc in range(GPCOL):
                    nsl = 2 * KG
                    it = sw.tile([128, nsl // 16], i16, tag="idx")
                    s0 = pcc * nsl
                    nc.sync.dma_start(
                        out=it[:, :], in_=gsidx[:, s0 // 16:(s0 + nsl) // 16])
                    g = sw.tile([128, nsl // 128, 64], f32, tag="gat", bufs=3)
                    nc.gpsimd.dma_gather(
                        out_ap=g[:, :, :], in_ap=m_my[:, :], idxs_ap=it[:, :],
                        num_idxs=nsl, num_idxs_reg=nsl, elem_size=64,
                        single_packet=False, queue_num=qn())
                    fm = psp.tile([128, KG], f32, tag="fm")
                    npc = nsl // 128
                    gp = g[:, :, :].rearrange("p a b -> p (a b)")
                    for i in range(npc // 2):
                        nc.tensor.transpose(
                            out=fm[:, 128 * i:128 * i + 128],
                            in_=gp[:, 128 * i:128 * i + 128],
                            identity=ident[:, :])
                    if phase == 0:
                        for st, op in (("mn", ALU.min), ("mx", ALU.max),
                                       ("sm", ALU.add)):
                            nc.vector.tensor_reduce(
                                out=gstat[st][:, pcc:pcc + 1],
                                in_=fm[:, None, :], axis=AX.X, op=op)
                        nc.vector.tensor_copy(gfirst[:, pcc:pcc + 1],
                                              fm[:, 0:1])
                    else:
                        rl = sw.tile([128, KG], f32, tag="relu")
                        bsl = gbias[:, pcc:pcc + 1, None] \
                            .to_broadcast([128, 1, KG])
                        nc.vector.tensor_tensor(
                            out=rl[:, None, :], in0=fm[:, None, :], in1=bsl,
                            op=ALU.subtract)
                        nc.vector.tensor_scalar(
                            out=rl[:, :], in0=rl[:, :], scalar1=0.0,
                            scalar2=None, op0=ALU.max)
                        nc.vector.tensor_reduce(
                            out=gstat["rs"][:, pcc:pcc + 1],
                            in_=rl[:, None, :], axis=AX.X, op=ALU.add)
                        nc.vector.tensor_copy(gfirst[:, pcc:pcc + 1],
                                              rl[:, 0:1])
                gfw = smp.tile([128, GPCOL], f32, tag="g_fw")
                nc.vector.tensor_tensor(out=gfw[:, :], in0=gfirst[:, :],
                                        in1=gpad_t[:, :], op=ALU.mult)
                stn = "sm" if phase == 0 else "rs"
                nc.vector.tensor_tensor(out=gstat[stn][:, :],
                                        in0=gstat[stn][:, :], in1=gfw[:, :],
                                        op=ALU.subtract)
                if phase == 0:
                    nc.vector.tensor_tensor(
                        out=gbias[:, :], in0=gstat["mx"][:, :],
                        in1=gstat["mn"][:, :], op=ALU.subtract)
                    nc.vector.tensor_scalar(
                        out=gbias[:, :], in0=gbias[:, :],
                        scalar1=gtb_t[:, 0:1], scalar2=None, op0=ALU.mult)
                    nc.vector.tensor_tensor(
                        out=gbias[:, :], in0=gbias[:, :],
                        in1=gstat["mn"][:, :], op=ALU.add)
            emb = smp.tile([128, GPCOL], f32, tag="g_emb")
            nc.vector.tensor_scalar(out=emb[:, :], in0=gstat["mn"][:, :],
                                    scalar1=float(gwp[1]), scalar2=None,
                                    op0=ALU.mult)
            for st, wgt in (("mx", gwp[2]), ("rs", gwp[3]), ("sm", gwp[4])):
                nc.vector.scalar_tensor_tensor(
                    out=emb[:, :], in0=gstat[st][:, :], scalar=float(wgt),
                    in1=emb[:, :], op0=ALU.mult, op1=ALU.add)
            nc.vector.tensor_tensor(out=emb[:, :], in0=emb[:, :],
                                    in1=c0g_t[:, :], op=ALU.add)
            e65 = smp.tile([65, 2 * GPCOL], f32, tag="e65")
            nc.vector.memset(e65[64:65, :], 1.0)
            nc.sync.dma_start(out=e65[0:64, 0:GPCOL], in_=emb[0:64, :])
            nc.sync.dma_start(out=e65[0:64, GPCOL:2 * GPCOL],
                              in_=emb[64:128, :])
            pso = ps1.tile([2 * GPCOL, 32], f32, tag="ps_small")
            nc.tensor.matmul(out=pso[:, :], lhsT=e65[:, :], rhs=wout_t[:, :],
                             start=True, stop=True)
            ot = smp.tile([2 * GPCOL, 32], f32, tag="ot")
            nc.vector.tensor_copy(ot[:, :], pso[:, :])
            nc.sync.dma_start(out=out_ext[:, :], in_=ot[:, :])
    nc.compile()
    return nc


# ============================================================ input prep
def make_inputs(plan, inputs):
    x = np.asarray(inputs["x"], np.float32)
    Wlin = np.asarray(inputs["Wlin"], np.float32)
    blin = np.asarray(inputs["blin"], np.float32)
    t = np.clip(np.asarray(inputs["t"], np.float32), 0, 1)
    Wc = np.asarray(inputs["Wc"], np.float32)
    bc = np.asarray(inputs["bc"], np.float32)
    gW = np.asarray(inputs["gWlin"], np.float32)
    gb = np.asarray(inputs["gblin"], np.float32)
    gt = np.clip(np.asarray(inputs["gt"], np.float32), 0, 1)
    gWp = np.asarray(inputs["gWproj"], np.float32)
    Wout = np.asarray(inputs["Wout"], np.float32)
    bout = np.asarray(inputs["bout"], np.float32)

    plan.consts = dict(
        Wproj=np.asarray(inputs["Wproj"], np.float32),
        bproj=np.asarray(inputs["bproj"], np.float32),
        gWp=gWp, gbp=float(np.asarray(inputs["gbproj"], np.float32)))

    wincols = [int(plan.win_paircol0[w + 1] - plan.win_paircol0[w])
               for w in range(NWIN)]
    PCW = [-(-wc // 128) * 128 for wc in wincols]

    def wrap_chunks(arr, chunk_sizes):
        outs, o = [], 0
        for ns in chunk_sizes:
            outs.append(wrap_idx16(arr[o:o + ns]))
            o += ns
        return np.concatenate(outs, 1)

    chunk_sizes = [2 * ncols * K for (w, b, K, s0, pc0, ncols) in plan.chunks]
    gchunk_sizes = [2 * plan.KG] * plan.GPCOL

    shared = {}
    shared["wouta"] = np.concatenate([Wout.T, bout[None, :]], 0).astype(np.float32)
    shared["gwa"] = np.concatenate([gW.T, gb[None, :]], 0).astype(np.float32)
    shared["gtb"] = np.concatenate([gt, gt])[:, None].astype(np.float32)
    for l in range(L):
        shared[f"wlin{l}"] = np.concatenate(
            [Wlin[l].T, blin[l][None, :]], 0).astype(np.float32)
        shared[f"wc{l}"] = np.ascontiguousarray(Wc[l].T)
        shared[f"bc{l}"] = np.ascontiguousarray(bc[l][:, None])
        shared[f"tb{l}"] = np.tile(t[l][None, :], (128, 1))

    in_maps = []
    nrow = plan.nrow
    for c in range(M):
        im = dict(shared)
        xc = np.zeros((NC, 64), np.float32)
        nr = plan.n_real[c]
        xc[:nr] = x[plan.bounds[c]:plan.bounds[c + 1]]
        im["x"] = np.ascontiguousarray(xc.T)
        im["sidx"] = wrap_chunks(plan.slot_idx[c], chunk_sizes)
        for w in range(NWIN):
            im[f"cidx{w}"] = wrap_idx16(plan.comb_idx[(c, w)])
        ba, bb_ = [], []
        for w in range(NWIN):
            p0, p1 = int(plan.win_paircol0[w]), int(plan.win_paircol0[w + 1])
            a = np.zeros(PCW[w], np.int64)
            b_ = np.zeros(PCW[w], np.int64)
            a[:p1 - p0] = nrow[plan.biasA_idx[c, p0:p1]]
            b_[:p1 - p0] = nrow[plan.biasB_idx[c, p0:p1]]
            ba.append(wrap_idx16(a))
            bb_.append(wrap_idx16(b_))
        im["baidx"] = np.concatenate(ba, 1)
        im["bbidx"] = np.concatenate(bb_, 1)
        im["gsidx"] = wrap_chunks(plan.gslot_idx[c], gchunk_sizes)
        pf = np.zeros((128, plan.PC), np.float32)
        pf[0:64, :] = plan.pad_arr[c, 0][None, :]
        pf[64:128, :] = plan.pad_arr[c, 1][None, :]
        im["padf"] = pf
        im["cntb"] = np.ascontiguousarray(plan.cnt[c].reshape(NT, 128).T)
        gp = np.zeros((128, plan.GPCOL), np.float32)
        gp[0:64, :] = plan.gpad[c, 0][None, :]
        gp[64:128, :] = plan.gpad[c, 1][None, :]
        im["gpadt"] = gp
        c0gv = gWp[0] * plan.gcnt[c] + plan.consts["gbp"]   # [2, GPCOL]
        cg = np.zeros((128, plan.GPCOL), np.float32)
        cg[0:64, :] = c0gv[0][None, :]
        cg[64:128, :] = c0gv[1][None, :]
        im["c0g"] = cg
        in_maps.append(im)
    return in_maps


def kernel(**inputs):
    plan = Plan(np.asarray(inputs["edge_index"]), np.asarray(inputs["batch"]))
    in_maps = make_inputs(plan, inputs)
    nc = build_nc(plan)
    from concourse.bass_utils import run_bass_kernel_spmd
    r = run_bass_kernel_spmd(nc, in_maps, core_ids=list(range(M)))
    outs = [r.results[c]["out"][:plan.gnum[c]] for c in range(M)]
    out = np.concatenate(outs, 0).astype(np.float32)
    assert out.shape == (G, OUT)
    return out

